# revision 27
# baseline (speedup 1.0000x reference)
"""Pristine baseline kernel (422-470us HW) reconstructed verbatim as fallback."""

import sys

sys.path.insert(0, "/opt/trn_rl_repo")

import numpy as np
import ml_dtypes

import concourse.bass as bass  # noqa: F401
import concourse.mybir as mybir
import concourse.tile as tile
from concourse import bacc
from concourse.bass_utils import run_bass_kernel_spmd

BF16 = mybir.dt.bfloat16
F32 = mybir.dt.float32
AF = mybir.ActivationFunctionType
OP = mybir.AluOpType

B, T, U = 2048, 200, 128
NCORES = 8
BL = B // NCORES
TC = 25
NCHUNK = T // TC

PROFILE = False
LAST_RESULT = None
LAST_IN_MAPS = None

_cache = {}


def _build(has_brz: bool, T_=T, TC_=TC, BL_=BL, reps=1):
    NCHUNK_ = T_ // TC_
    nc = bacc.Bacc("TRN2", target_bir_lowering=False)

    xt = nc.dram_tensor("xt", [U, T_, BL_], BF16, kind="ExternalInput")
    av = nc.dram_tensor("av", [T_ * BL_], BF16, kind="ExternalInput")
    h0t = nc.dram_tensor("h0t", [U, BL_], BF16, kind="ExternalInput")
    wcat = nc.dram_tensor("wcat", [6, U, U], BF16, kind="ExternalInput")
    ident_d = nc.dram_tensor("ident", [U, U], BF16, kind="ExternalInput")
    biases = nc.dram_tensor("biases", [U, 3], F32, kind="ExternalInput")
    ones_d = nc.dram_tensor("ones1", [1, U], BF16, kind="ExternalInput")
    outt = nc.dram_tensor("outt", [U, T_, BL_], BF16, kind="ExternalOutput")

    with tile.TileContext(nc) as tc:
        with (
            tc.tile_pool(name="const", bufs=1) as cpool,
            tc.tile_pool(name="xchunk", bufs=2) as xpool,
            tc.tile_pool(name="achunk", bufs=2) as apool,
            tc.tile_pool(name="abc", bufs=2) as abcpool,
            tc.tile_pool(name="ochunk", bufs=2) as opool,
            tc.tile_pool(name="work", bufs=4) as wpool,
            tc.tile_pool(name="psum", bufs=2, space="PSUM") as ppool,
        ):
            wts = []
            for i in range(6):
                wt = cpool.tile([U, U], BF16, tag=f"w{i}")
                nc.sync.dma_start(wt[:], wcat[i])
                wts.append(wt)
            w_r, u_r, w_z, u_z, w_h, u_h = wts
            ident = cpool.tile([U, U], BF16, tag="ident")
            nc.sync.dma_start(ident[:], ident_d[:])
            ones1 = cpool.tile([1, U], BF16, tag="ones1")
            nc.sync.dma_start(ones1[:], ones_d[:])
            btile = cpool.tile([U, 3], F32, tag="biases")
            nc.sync.dma_start(btile[:], biases[:])
            b_r_ap = btile[:, 0:1]
            b_z_ap = btile[:, 1:2]
            b_h_ap = btile[:, 2:3]
            h0tile = cpool.tile([U, BL_], BF16, tag="h0")
            nc.sync.dma_start(h0tile[:], h0t[:])

            for _rep in range(reps):
                xchs = {}

                def load_chunk(k):
                    if k >= NCHUNK_ or k in xchs:
                        return
                    t0, t1x = k * TC_, (k + 1) * TC_
                    xch = xpool.tile([U, TC_, BL_], BF16, tag="xch")
                    nc.sync.dma_start(xch[:], xt[:, t0:t1x, :])
                    ach = apool.tile([1, TC_ * BL_], BF16, tag="ach")
                    nc.sync.dma_start(ach[:], av[t0 * BL_ : t1x * BL_])
                    xchs[k] = (xch, ach)

                def emit_xside(t):
                    k, dt = divmod(t, TC_)
                    xch, ach = xchs[k]
                    xs = xch[:, dt, :]
                    ps_rz = ppool.tile([U, 2 * BL_], F32, tag="ps_rz")
                    nc.tensor.matmul(ps_rz[:, 0:BL_], w_r[:], xs, start=True, stop=False)
                    nc.tensor.matmul(ps_rz[:, BL_:], w_z[:], xs, start=False, stop=False)
                    ps_xh_full = ppool.tile([U, 2 * BL_], F32, tag="ps_xh")
                    ps_xh = ps_xh_full[:, 0:BL_]
                    nc.tensor.matmul(ps_xh, w_h[:], xs, start=True, stop=False)
                    ps_a = ppool.tile([U, BL_], F32, tag="ps_a")
                    nc.tensor.matmul(
                        ps_a[:], ones1[:], ach[:, dt * BL_ : (dt + 1) * BL_],
                        start=True, stop=True,
                    )
                    return ps_rz, ps_xh, ps_a

                h_prev = h0tile[:]
                load_chunk(0)
                pending = emit_xside(0)
                och = None
                for t in range(T_):
                    k, dt = divmod(t, TC_)
                    if dt == 0:
                        load_chunk(k + 1)
                        och = opool.tile([U, TC_, BL_], BF16, tag="och")
                    ps_rz, ps_xh, ps_a = pending

                    ps_mmh = ppool.tile([U, BL_], F32, tag="ps_mmh")
                    nc.tensor.matmul(ps_mmh[:], u_h[:], h_prev, start=True, stop=True)
                    nc.tensor.matmul(ps_rz[:, BL_:], u_z[:], h_prev, start=False, stop=False)
                    nc.tensor.matmul(ps_rz[:, 0:BL_], u_r[:], h_prev, start=False, stop=True)

                    r_sb = wpool.tile([U, BL_], BF16, tag="r_sb")
                    if has_brz:
                        nc.scalar.activation(r_sb[:], ps_rz[:, 0:BL_], AF.Sigmoid, bias=b_r_ap)
                    else:
                        nc.scalar.activation(r_sb[:], ps_rz[:, 0:BL_], AF.Sigmoid)
                    u_sb = wpool.tile([U, BL_], BF16, tag="u_sb")
                    if has_brz:
                        nc.scalar.activation(u_sb[:], ps_rz[:, BL_:], AF.Sigmoid, bias=b_z_ap)
                    else:
                        nc.scalar.activation(u_sb[:], ps_rz[:, BL_:], AF.Sigmoid)

                    t1 = wpool.tile([U, BL_], BF16, tag="t1")
                    nc.vector.tensor_tensor(t1[:], ps_mmh[:], r_sb[:], OP.mult)
                    nc.tensor.matmul(ps_xh, ident[:], t1[:], start=False, stop=True)
                    if t + 1 < T_:
                        pending = emit_xside(t + 1)

                    uhat = wpool.tile([U, BL_], BF16, tag="uhat")
                    nc.vector.tensor_tensor(uhat[:], u_sb[:], ps_a[:], OP.mult)
                    m1 = wpool.tile([U, BL_], BF16, tag="m1")
                    nc.vector.scalar_tensor_tensor(
                        m1[:], uhat[:], 1.0, h_prev, OP.subtract, OP.mult
                    )
                    htil = wpool.tile([U, BL_], BF16, tag="htil")
                    nc.scalar.activation(htil[:], ps_xh, AF.Tanh, bias=b_h_ap)
                    m2 = wpool.tile([U, BL_], BF16, tag="m2")
                    nc.vector.tensor_tensor(m2[:], uhat[:], htil[:], OP.mult)
                    hn = och[:, dt, :]
                    nc.vector.tensor_tensor(hn, m2[:], m1[:], OP.subtract)
                    h_prev = hn

                    if dt == TC_ - 1:
                        nc.sync.dma_start(outt[:, k * TC_ : (k + 1) * TC_, :], och[:])
                        xchs.pop(k, None)

    nc.compile()
    return nc


def kernel(inputs, h0, W_r, U_r, b_r, W_z, U_z, b_z, W_h, U_h, b_h):
    global LAST_RESULT
    inputs = np.asarray(inputs, dtype=np.float32)
    h0 = np.asarray(h0, dtype=np.float32)
    ws = [np.asarray(w, dtype=np.float32) for w in (W_r, U_r, W_z, U_z, W_h, U_h)]
    bs = [np.asarray(b, dtype=np.float32) for b in (b_r, b_z, b_h)]

    has_brz = bool(np.any(bs[0]) or np.any(bs[1]))
    key = has_brz
    if key not in _cache:
        _cache[key] = _build(has_brz)
    nc = _cache[key]

    bf = ml_dtypes.bfloat16
    wcat = np.stack([w.astype(bf) for w in ws])
    ident = np.eye(U, dtype=bf)
    ones1 = np.ones((1, U), dtype=bf)
    biases = np.stack([bs[0], bs[1], bs[2]], axis=1).astype(np.float32)

    x = inputs[:, :, :U]
    a = inputs[:, :, U]

    in_maps = []
    for c in range(NCORES):
        sl = slice(c * BL, (c + 1) * BL)
        xt_c = np.ascontiguousarray(x[sl].transpose(2, 1, 0)).astype(bf)
        a_c = np.ascontiguousarray(a[sl].T).astype(bf).reshape(T * BL)
        h0t_c = np.ascontiguousarray(h0[sl].T).astype(bf)
        in_maps.append(
            {
                "xt": xt_c,
                "av": a_c,
                "h0t": h0t_c,
                "wcat": wcat,
                "ident": ident,
                "biases": biases,
                "ones1": ones1,
            }
        )

    res = run_bass_kernel_spmd(nc, in_maps, list(range(NCORES)), trace=PROFILE)
    global LAST_IN_MAPS
    LAST_IN_MAPS = in_maps
    LAST_RESULT = res

    out = np.empty((B, T, U), dtype=np.float32)
    for c in range(NCORES):
        sl = slice(c * BL, (c + 1) * BL)
        out[sl] = res.results[c]["outt"].astype(np.float32).transpose(2, 1, 0)
    return out
